# revision 49
# baseline (speedup 1.0000x reference)
"""BiSPA (bidirectional sparse windowed attention + MLP) Trainium2 kernel.

Full inputs in, full outputs out. Internally shards across 8 NeuronCores:
core c owns output rows Ic = [24c, 24c+24) of the (192, 192, 512) grid.

Key observation: with B == S == 192 and window W == 32,
  - vertical attention for output row i is a complete 192-token sliding-window
    attention over x[i, :, :]                        (needs x rows  Ic)
  - horizontal attention for output row i is a complete 192-token
    sliding-window attention with Q from x[i, :, :] and K/V from x[:, i, :]
    (needs x columns Ic)
so each core needs x[Ic, :, :] and x[:, Ic, :] and NOTHING else -> zero
duplicated projection FLOPs, zero collectives, no halos.

v2 changes vs the first working kernel (tensor-engine bound at ~864us):
  - out-projections folded into MLP1 on the host: F_h = W1h @ Who,
    F_v = W1v @ Wvo, c1 = W1h@h_out_eff + W1v@v_out_eff + b1. Removes
    384 N=384 matmuls + 8 evictions per pair.
  - scores for a head pair land in one 2-bank PSUM tile -> ONE exp
    activation per head pair (batched [128, 2, 224] strided read).
  - softmax normalization: one batched reciprocal per head pair
    ([128,2,2,1] AP over the 4 Z columns), 4 DVE scale ops.
  - ctx^T transposes stay on the PE but write f32 into SPARE COLUMNS
    (260:452) of the same PSUM bank as the attn@V accumulator, so no
    extra PSUM banks are needed; ONE ACT copy per head pair evicts.
  - emission software-pipelined: QK projections of pair g+1 interleave
    with the attention inner loop of pair g so the PE array duty never
    dips long enough for HAM to re-throttle the clock (354us of the
    baseline ran at 1.2 GHz instead of 2.4); MLP of pair g is emitted
    during pair g+1 so it never waits on fresh ctx evictions.

Numerics: all matmul inputs bf16, fp32 PSUM accumulation, fp32 softmax exp
input, bf16 probs/ctx.  Measured end-to-end rel err vs fp32 reference ~0.6%.
"""

import numpy as np
from contextlib import ExitStack

import concourse.bass as bass
import concourse.mybir as mybir
import concourse.tile as tile
from concourse import bacc
from concourse.bass_utils import run_bass_kernel_spmd
from concourse.masks import make_identity
from concourse.tile import add_dep_helper


def _chain(insts):
    """Order matmuls targeting one PSUM bank: a start=True zeroes (marks
    pending-zero) the WHOLE 2KB bank, so each bank must hold exactly one
    accumulation group and the group's matmuls must execute in program order.
    Tile won't order disjoint-region writes by itself."""
    for a, b in zip(insts, insts[1:]):
        add_dep_helper(b.ins, a.ins, sync=False, reason="psum-bank group order")

BF = mybir.dt.bfloat16
F32 = mybir.dt.float32
AF = mybir.ActivationFunctionType
MUL = mybir.AluOpType.mult
NPBF = mybir.dt.np(BF)

E = 512
H = 8
D = 64
W = 32
S = 192
NCORE = 8
RPC = 24          # rows (strips) per core
T = RPC * S       # tokens per core per branch = 4608
NPAIR = RPC // 2  # strip pairs per core


def _band_masks():
    """Score mask, bf16 (128, 384): [TA 96 | TB 96] x 2 heads.

    TA: rows p = key k in [0,128), cols q in [0,96).
        valid = |k-q| <= W              (every key for q<96 is in [0,128))
    TB: rows p -> key k = 64+p in [64,192), cols q' -> q = 96+q' in [96,192).
        valid = |k-q| <= W              (every key for q>=96 is in [64,192))

    q-ranges are disjoint between TA and TB, so the three attn@V matmuls
    per head write DISJOINT PSUM cells (no accumulate-overlap -> the PE
    pipelines them instead of serializing on the drain).
    """
    m = np.zeros((128, 192), np.float32)
    k = np.arange(128)[:, None]
    q = np.arange(96)[None, :]
    m[:, 0:96] = np.abs(k - q) <= W
    kb = 64 + np.arange(128)[:, None]
    qb = 96 + np.arange(96)[None, :]
    m[:, 96:192] = np.abs(kb - qb) <= W
    m2 = np.concatenate([m, m], axis=1)          # two heads of one p
    # [p0: 384 | pad 128 | p1: 384 | pad 128] matching the [128, 2, 512]
    # probs tiles, so ONE masked-multiply covers a whole head-pair-pair
    z = np.zeros((128, 128), np.float32)
    return np.concatenate([m2, z, m2, z], axis=1).astype(NPBF)


def _build_program(bias_flags):
    """Build the SPMD Bass/Tile program (same program on all 8 cores)."""
    has_vqk_b, has_hq_b, has_hk_b, has_c1, has_b2 = bias_flags

    nc = bacc.Bacc("TRN2", target_bir_lowering=False, debug=False,
                   num_devices=NCORE, num_swdge_queues=4)

    xr_t = nc.dram_tensor("xr_t", [E, T], BF, kind="ExternalInput").ap()
    xc_t = nc.dram_tensor("xc_t", [E, T], BF, kind="ExternalInput").ap()
    w_vin = nc.dram_tensor("w_vin", [E, 3 * E], BF, kind="ExternalInput").ap()
    w_hq = nc.dram_tensor("w_hq", [E, E], BF, kind="ExternalInput").ap()
    w_hkv = nc.dram_tensor("w_hkv", [E, 2 * E], BF, kind="ExternalInput").ap()
    w_fh = nc.dram_tensor("w_fh", [E, E], BF, kind="ExternalInput").ap()
    w_fv = nc.dram_tensor("w_fv", [E, E], BF, kind="ExternalInput").ap()
    w_m2 = nc.dram_tensor("w_m2", [E, E], BF, kind="ExternalInput").ap()
    mask_d = nc.dram_tensor("mask", [128, 1024], BF, kind="ExternalInput").ap()
    bias_d = nc.dram_tensor("biases", [128, 32], F32, kind="ExternalInput").ap()
    out_t = nc.dram_tensor("out_t", [E, T], F32, kind="ExternalOutput").ap()

    with tile.TileContext(nc) as tc, ExitStack() as ctx:
        pw = ctx.enter_context(tc.tile_pool(name="pw", bufs=1))
        psA = ctx.enter_context(tc.tile_pool(name="psA", bufs=2, space="PSUM"))
        psS = ctx.enter_context(tc.tile_pool(name="psS", bufs=2, space="PSUM"))
        psC = ctx.enter_context(tc.tile_pool(name="psC", bufs=2, space="PSUM"))
        px = ctx.enter_context(tc.tile_pool(name="px", bufs=3))
        pqk = ctx.enter_context(tc.tile_pool(name="pqk", bufs=32))
        pv = ctx.enter_context(tc.tile_pool(name="pv", bufs=8))
        pp = ctx.enter_context(tc.tile_pool(name="pp", bufs=12))
        pctx = ctx.enter_context(tc.tile_pool(name="pctx", bufs=8))
        pzr = ctx.enter_context(tc.tile_pool(name="pzr", bufs=8))
        pct = ctx.enter_context(tc.tile_pool(name="pct", bufs=16))
        phid = ctx.enter_context(tc.tile_pool(name="phid", bufs=8))
        pout = ctx.enter_context(tc.tile_pool(name="pout", bufs=8))

        # ---- persistent constants (loaded lazily in the emission section
        # so pair-0's activations and the QK-path weights reach SBUF first,
        # split across the gpsimd and sync DMA queues) ----
        def load_const(name, dram_ap, shape, dtype, eng):
            t = pw.tile(shape, dtype, tag=name)
            eng.dma_start(t[:], dram_ap)
            return t

        ident = pw.tile([128, 128], BF, tag="ident")
        make_identity(nc, ident)

        # bias column map (within `bia`):
        # 0-7 v_in_b[0:1024] ftiles; 8-11 h_in_b[0:512]; 12-15 h_in_b[512:1024]
        # 24-27 c1 (fused W1@out_bias + mlp_b1); 28-31 mlp_b2

        # per-pair state, filled by the emit helpers
        xr2_all = [None] * NPAIR
        xc2_all = [None] * NPAIR
        qk_all = [None] * NPAIR
        ct_all = [None] * NPAIR
        # ACT evictions of past cp tiles (psC bufs=2): the bitcast APs used
        # by the transposes/evict may not be seen by Tile's overlap tracker,
        # so the next occupant of the same PSUM buffer syncs explicitly.
        cp_evicts = []

        def emit_dma(g):
            g0 = 2 * S * g
            xr2, xc2 = [], []
            for k in range(4):
                t = px.tile([128, 2 * S], BF, tag=f"xr{k}")
                nc.gpsimd.dma_start(t[:], xr_t[128 * k:128 * (k + 1), g0:g0 + 2 * S])
                xr2.append(t)
                t = px.tile([128, 2 * S], BF, tag=f"xc{k}")
                nc.sync.dma_start(t[:], xc_t[128 * k:128 * (k + 1), g0:g0 + 2 * S])
                xc2.append(t)
            xr2_all[g] = xr2
            xc2_all[g] = xc2

        def emit_qk_ftile(g, branch, j):
            """One QK projection ftile for pair g, feature-major, N=384.
            ftile j in 0..7: j<4 -> Q features, j>=4 -> K features."""
            xr2, xc2 = xr2_all[g], xc2_all[g]
            ps = psA.tile([128, 384], F32, tag="proj",
                          padded_shape=[128, 512])
            for k in range(4):
                if branch == "v":
                    lhsT = wv[k][:, 128 * j:128 * (j + 1)]
                    rhs = xr2[k][:]
                elif j < 4:   # h Q
                    lhsT = whq[k][:, 128 * j:128 * (j + 1)]
                    rhs = xr2[k][:]
                else:         # h K
                    lhsT = whkv[k][:, 128 * (j - 4):128 * (j - 3)]
                    rhs = xc2[k][:]
                nc.tensor.matmul(ps[:], lhsT=lhsT, rhs=rhs,
                                 start=(k == 0), stop=(k == 3))
            # bias columns: v ftiles 0-7 -> cols 0-7; h Q 0-3 -> 8-11;
            # h K 0-3 -> 12-15. Evictions alternate ACT/DVE so neither
            # engine's FIFO latency stalls the psA buffer rotation.
            dst = pqk.tile([128, 384], BF, tag="qk")
            need_b = (has_vqk_b if branch == "v"
                      else (has_hq_b if j < 4 else has_hk_b))
            if need_b:
                bcol = j if branch == "v" else (8 + j)
                nc.scalar.activation(dst[:], ps[:], AF.Identity,
                                     bias=bia[:, bcol:bcol + 1])
            elif j % 2 == 0:
                nc.scalar.activation(dst[:], ps[:], AF.Copy)
            else:
                nc.vector.tensor_copy(dst[:], ps[:])
            qk_all[g][branch][j] = dst

        def emit_qk(g, branch):
            if qk_all[g] is None:
                qk_all[g] = {}
            qk_all[g][branch] = [None] * 8
            for j in range(8):
                emit_qk_ftile(g, branch, j)

        def qk_chunks(g):
            """8 chunk-emitters covering pair g's QK projections (2 ftiles
            = 8 matmuls each), to be interleaved into pair g-1's attention
            at the points where the PE would otherwise stall on the DVE
            normalize latency."""
            if qk_all[g] is None:
                qk_all[g] = {}
            chunks = []
            for branch in ("h", "v"):
                qk_all[g][branch] = [None] * 8
                for jj in (0, 2, 4, 6):
                    def mk(branch=branch, jj=jj):
                        emit_qk_ftile(g, branch, jj)
                        emit_qk_ftile(g, branch, jj + 1)
                    chunks.append(mk)
            return chunks

        def emit_attn_sb(g, a, br, feed):
            """Attention for strip a (0/1) of pair g, branch br. `feed()`
            emits a chunk of dense matmuls at the two points where the PE
            stream would otherwise stall on DVE normalize latency."""
            s0 = S * a
            xin = xr2_all[g] if br == "v" else xc2_all[g]
            vcols = slice(1024, 1536) if br == "v" else slice(512, 1024)
            vw = wv if br == "v" else whkv
            qk = qk_all[g][br]
            ct = ct_all[g]

            # ------ V projection, token-major, with ones column ----
            vps_a = psA.tile([128, 512], F32, tag="proj")
            vps_b = psA.tile([128, 512], F32, tag="proj")
            for k in range(4):
                nc.tensor.matmul(vps_a[:], lhsT=xin[k][:, s0:s0 + 128],
                                 rhs=vw[k][:, vcols],
                                 start=(k == 0), stop=(k == 3))
            for k in range(4):
                nc.tensor.matmul(vps_b[:], lhsT=xin[k][:, s0 + 64:s0 + 192],
                                 rhs=vw[k][:, vcols],
                                 start=(k == 0), stop=(k == 3))
            va = pv.tile([128, 8, 65], BF, tag="vp")   # keys [0:128)
            vb = pv.tile([128, 8, 65], BF, tag="vp")   # keys [64:192)
            nc.vector.tensor_copy(
                va[:, :, 0:64],
                vps_a[:].rearrange("p (h c) -> p h c", c=64))
            nc.vector.tensor_copy(
                vb[:, :, 0:64],
                vps_b[:].rearrange("p (h c) -> p h c", c=64))
            nc.vector.memset(va[:, :, 64:65], 1.0)
            nc.vector.memset(vb[:, :, 64:65], 1.0)

            # ------ scores + exp + mask, head pairs processed two at a
            # time sharing one probs tile -> ONE masked-multiply per four
            # heads ------
            pms = []
            for pg2 in (0, 2):
                pbp = pp.tile([128, 2, 512], BF, tag="p")
                pmp = pp.tile([128, 2, 512], BF, tag="p")
                for i, p in enumerate((pg2, pg2 + 1)):
                    QT = qk[p][:, s0:s0 + S]
                    KT = qk[4 + p][:, s0:s0 + S]
                    # one 2-bank tile per head pair; head h2 -> bank h2.
                    # Matmuls with disjoint contraction row-groups (head0
                    # at partitions 0:64, head1 at 64:128) run CONCURRENTLY
                    # on the PE and hard-fault if they write the same bank.
                    sp = psS.tile([128, 2, 512], F32, tag="sc")
                    for h2 in range(2):
                        d0 = 64 * h2
                        nc.tensor.matmul(sp[:, h2:h2 + 1, 0:96],
                                         lhsT=KT[d0:d0 + 64, 0:128],
                                         rhs=QT[d0:d0 + 64, 0:96],
                                         start=True, stop=True)
                        nc.tensor.matmul(sp[:, h2:h2 + 1, 96:192],
                                         lhsT=KT[d0:d0 + 64, 64:192],
                                         rhs=QT[d0:d0 + 64, 96:192],
                                         start=True, stop=True)
                    # ONE exp for both heads: strided [128, 2, 192] read
                    nc.scalar.activation(
                        pbp[:, i:i + 1, 0:384]
                        .rearrange("p a (b c) -> p (a b) c", b=2),
                        sp[:, :, 0:192], AF.Exp, scale=0.125)
                nc.vector.tensor_tensor(pmp[:, :, 0:384], pbp[:, :, 0:384],
                                        msk[:, 0:1024]
                                        .rearrange("p (i c) -> p i c", i=2)
                                        [:, :, 0:384], op=MUL)
                pms.append((pmp, 0))
                pms.append((pmp, 1))

            # ------ attn@V + normalize + ctx transpose per head pair ------
            # p-groups are processed two at a time with their matmuls
            # interleaved across the two psC banks so drains overlap.
            for pg in (0, 2):
                group = []
                for p in (pg, pg + 1):
                    pmp, pi = pms[p]
                    # attn@V: TWO matmuls per head (q 0:96 via va, q 96:192
                    # via vb), writing DISJOINT cells of one PSUM bank, all
                    # within partitions 0:96. The built-in group checker
                    # cannot express multi-region banks, so skip it;
                    # correctness comes from the issue ordering +
                    # per-element pending-zero semantics (mm1's start=True
                    # clears the bank rows this group ever touches).
                    cp = psC.tile([128, 512], F32, tag="cx")
                    mms = []
                    for h2 in range(2):
                        h = 2 * p + h2
                        cb = 130 * h2
                        ta = 192 * h2
                        tb = 192 * h2 + 96
                        # q in [0,96): keys [0:128) from TA
                        mms.append(nc.tensor.matmul(
                            cp[0:96, cb:cb + 65],
                            lhsT=pmp[:, pi:pi + 1, ta:ta + 96],
                            rhs=va[:, h:h + 1, :], start=(h2 == 0),
                            stop=False, skip_group_check=True))
                        # q in [96,192): keys [64:192) = all of TB
                        mms.append(nc.tensor.matmul(
                            cp[0:96, cb + 65:cb + 130],
                            lhsT=pmp[:, pi:pi + 1, tb:tb + 96],
                            rhs=vb[:, h:h + 1, :],
                            start=False, stop=(h2 == 1),
                            skip_group_check=True))
                    _chain(mms)
                    group.append((p, cp, mms))

                norm_done = []
                for p, cp, mms in group:
                    # normalize by 1/Z (Z = ones-column accumulation at cols
                    # {64, 129, 194, 259} = 64 + 130h + 65q) and pack for
                    # the transpose: ctxn = [q1: h0|h1, q2: h0|h1], 64 cols
                    # each, on partitions 0:96 (q1 = tok 0:96, q2 = 96:192)
                    zr = pzr.tile([96, 2, 2, 1], F32, tag="zr")
                    zin = (cp[0:96, 64:324]
                           .rearrange("p (h x) -> p h x", h=2)
                           .rearrange("p h (q c) -> p h q c", q=2))
                    ctxn = pctx.tile([96, 256], BF, tag="ctxn")
                    cpq = (cp[0:96, 0:260]
                           .rearrange("p (h x) -> p h x", h=2)
                           .rearrange("p h (q c) -> p h q c", q=2))
                    cto = ctxn[:].rearrange("p (q h c) -> p h q c", q=2, h=2)
                    reads = [
                        nc.vector.reciprocal(zr[:], zin[:, :, :, 0:1]),
                        nc.vector.tensor_tensor(
                            cto, cpq[:, :, :, 0:64],
                            zr[:, :, :, 0:1].broadcast_to([96, 2, 2, 64]),
                            op=MUL),
                    ]
                    # cp reads must wait for the accumulation group to close
                    # (same-bank PE-write + DVE-read is a HW fault)
                    for r in reads:
                        add_dep_helper(r.ins, mms[-1].ins, sync=True,
                                       reason="psum read after group close")
                    norm_done.append((p, cp, mms, ctxn, reads))

                # dense matmuls here: they keep the PE streaming while the
                # DVE normalize drains, so the transposes never stall the
                # PE FIFO (head-of-line).
                feed()

                for p, cp, mms, ctxn, reads in norm_done:
                    # ctx^T via PE transpose into the SPARE columns
                    # (260:356) of the same PSUM bank -- no extra banks.
                    tps = [
                        nc.tensor.transpose(cp[:, 260:308].bitcast(BF),
                                            ctxn[:, 0:128],
                                            ident[0:96, 0:96]),
                        nc.tensor.transpose(cp[:, 308:356].bitcast(BF),
                                            ctxn[:, 128:256],
                                            ident[0:96, 0:96]),
                    ]
                    for t in tps:
                        for r in reads:
                            add_dep_helper(t.ins, r.ins, sync=True,
                                           reason="transpose after reads")
                    ct_p = ct[(0 if br == "h" else 4) + p]
                    if p % 2 == 0:
                        ev = nc.scalar.activation(
                            ct_p[:, s0:s0 + S],
                            cp[:, 260:356].bitcast(BF), AF.Copy)
                    else:
                        ev = nc.vector.tensor_copy(
                            ct_p[:, s0:s0 + S],
                            cp[:, 260:356].bitcast(BF))
                    for t in tps:
                        add_dep_helper(ev.ins, t.ins, sync=True,
                                       reason="evict after transpose")
                    cp_evicts.append(ev)
                    if len(cp_evicts) >= 3:
                        add_dep_helper(mms[0].ins, cp_evicts[-3].ins,
                                       sync=True,
                                       reason="cp buffer reuse after evict")

        def emit_mlp_hid(g, j, hid):
            ct = ct_all[g]
            ps = psA.tile([128, 384], F32, tag="proj",
                          padded_shape=[128, 512])
            for k in range(4):
                nc.tensor.matmul(
                    ps[:], lhsT=wfh[k][:, 128 * j:128 * (j + 1)],
                    rhs=ct[k][:], start=(k == 0), stop=False)
            for k in range(4):
                nc.tensor.matmul(
                    ps[:], lhsT=wfv[k][:, 128 * j:128 * (j + 1)],
                    rhs=ct[4 + k][:], start=False, stop=(k == 3))
            dst = phid.tile([128, 384], BF, tag="hid")
            if has_c1:
                nc.scalar.activation(dst[:], ps[:], AF.Relu,
                                     bias=bia[:, 24 + j:24 + j + 1])
            else:
                nc.scalar.activation(dst[:], ps[:], AF.Relu)
            hid[j] = dst

        def emit_mlp_out(g, j, hid):
            g0 = 2 * S * g
            ps = psA.tile([128, 384], F32, tag="proj",
                          padded_shape=[128, 512])
            for k in range(4):
                nc.tensor.matmul(ps[:],
                                 lhsT=wm2[k][:, 128 * j:128 * (j + 1)],
                                 rhs=hid[k][:],
                                 start=(k == 0), stop=(k == 3))
            osb = pout.tile([128, 384], F32, tag="o")
            if has_b2:
                nc.scalar.activation(osb[:], ps[:], AF.Identity,
                                     bias=bia[:, 28 + j:28 + j + 1])
            else:
                nc.scalar.activation(osb[:], ps[:], AF.Copy)
            nc.sync.dma_start(out_t[128 * j:128 * (j + 1), g0:g0 + 2 * S],
                              osb[:])

        def emit_mlp(g):
            """Fused (out-proj + MLP1) then MLP2 for pair g, N=384."""
            hid = [None] * 4
            for j in range(4):
                emit_mlp_hid(g, j, hid)
            for j in range(4):
                emit_mlp_out(g, j, hid)

        def mlp_chunks(g):
            """8 chunk-emitters covering pair g's MLP, for interleaving
            into the final pair's attention (which has no QK to feed)."""
            hid = [None] * 4
            chunks = []
            for j in range(4):
                chunks.append(lambda j=j: emit_mlp_hid(g, j, hid))
            for j in range(4):
                chunks.append(lambda j=j: emit_mlp_out(g, j, hid))
            return chunks

        # ---- software-pipelined emission ----
        # attention of pair g interleaves with QK projections of pair g+1
        # (dense N=384 matmuls) so the PE array duty stays high and HAM
        # keeps the 2.4 GHz clock.
        # whq rides the sync queue ahead of xc(0): the first matmuls (QK-h
        # ftiles 0-3) need only xr(0) [gpsimd] + whq [sync].
        whq = [load_const(f"whq{k}", w_hq[128 * k:128 * (k + 1), :],
                          [128, E], BF, nc.sync)
               for k in range(4)]
        emit_dma(0)
        whkv = [load_const(f"whkv{k}", w_hkv[128 * k:128 * (k + 1), :],
                           [128, 2 * E], BF,
                           nc.gpsimd if k % 2 == 0 else nc.sync)
                for k in range(4)]
        wv = [load_const(f"wv{k}", w_vin[128 * k:128 * (k + 1), :],
                         [128, 3 * E], BF,
                         nc.gpsimd if k % 2 == 0 else nc.sync)
              for k in range(4)]
        msk = load_const("msk", mask_d[:, :], [128, 1024], BF, nc.sync)
        bia = load_const("bia", bias_d[:, :], [128, 32], F32, nc.sync)
        emit_dma(1)
        wfh = [load_const(f"wfh{k}", w_fh[128 * k:128 * (k + 1), :],
                          [128, E], BF, nc.sync)
               for k in range(4)]
        wfv = [load_const(f"wfv{k}", w_fv[128 * k:128 * (k + 1), :],
                          [128, E], BF, nc.sync)
               for k in range(4)]
        wm2 = [load_const(f"wm2{k}", w_m2[128 * k:128 * (k + 1), :],
                          [128, E], BF, nc.sync)
               for k in range(4)]
        for g in range(NPAIR):
            ct_all[g] = [pct.tile([128, 2 * S], BF, tag="ct",
                                  name=f"ct_{g}_{i}") for i in range(8)]
            if g == 0:
                emit_qk(0, "h")
                emit_qk(0, "v")
            if g + 2 < NPAIR:
                emit_dma(g + 2)
            # For the last pair there is no QK to feed, so feed the
            # previous pair's MLP chunks into the stall points instead.
            last = g + 1 >= NPAIR
            chunks = mlp_chunks(g - 1) if last else qk_chunks(g + 1)
            ci = [0]

            def feed():
                if ci[0] < len(chunks):
                    chunks[ci[0]]()
                    ci[0] += 1
            for a in range(2):
                emit_attn_sb(g, a, "h", feed)
                emit_attn_sb(g, a, "v", feed)
                # MLP of the PREVIOUS pair: its ctx evictions are long done,
                # so these dense matmuls never stall the PE stream.
                if a == 1 and g > 0 and not last:
                    emit_mlp(g - 1)
        emit_mlp(NPAIR - 1)
    nc.finalize()
    return nc


_CACHE = {}


def _get_program(bias_flags):
    key = tuple(bias_flags)
    if key not in _CACHE:
        _CACHE[key] = _build_program(key)
    return _CACHE[key]


def _col(b):
    """bias vector (128*n,) -> (128, n) column-pack, fortran-ish layout."""
    return np.ascontiguousarray(b.reshape(-1, 128).T.astype(np.float32))


def kernel(hidden_states, h_in_w, h_in_b, h_out_w, h_out_b,
           v_in_w, v_in_b, v_out_w, v_out_b,
           mlp_w1, mlp_b1, mlp_w2, mlp_b2):
    x = np.asarray(hidden_states, dtype=np.float32)
    h_in_w = np.asarray(h_in_w, np.float32)
    h_in_b = np.asarray(h_in_b, np.float32)
    h_out_w = np.asarray(h_out_w, np.float32)
    h_out_b = np.asarray(h_out_b, np.float32)
    v_in_w = np.asarray(v_in_w, np.float32)
    v_in_b = np.asarray(v_in_b, np.float32)
    v_out_w = np.asarray(v_out_w, np.float32)
    v_out_b = np.asarray(v_out_b, np.float32)
    mlp_w1 = np.asarray(mlp_w1, np.float32)
    mlp_b1 = np.asarray(mlp_b1, np.float32)
    mlp_w2 = np.asarray(mlp_w2, np.float32)
    mlp_b2 = np.asarray(mlp_b2, np.float32)

    # V biases act as a constant shift of ctx (softmax weights sum to 1),
    # so fold them through the out-projections; then fold the
    # out-projections themselves into MLP1 (everything is linear up to the
    # ReLU): hid = relu(W1h(Who ctx_h + bho) + W1v(Wvo ctx_v + bvo) + b1)
    #            = relu(F_h ctx_h + F_v ctx_v + c1)
    h_out_eff = h_out_b + h_out_w @ h_in_b[2 * E:3 * E]
    v_out_eff = v_out_b + v_out_w @ v_in_b[2 * E:3 * E]
    w1h = mlp_w1[:, 0:E]
    w1v = mlp_w1[:, E:2 * E]
    f_h = w1h @ h_out_w          # (E, E): hid += F_h @ ctx_h
    f_v = w1v @ v_out_w
    c1 = w1h @ h_out_eff + w1v @ v_out_eff + mlp_b1

    bias_flags = (
        bool(np.any(v_in_b[0:2 * E])), bool(np.any(h_in_b[0:E])),
        bool(np.any(h_in_b[E:2 * E])), bool(np.any(c1)), bool(np.any(mlp_b2)),
    )
    nc = _get_program(bias_flags)

    biases = np.zeros((128, 32), np.float32)
    biases[:, 0:8] = _col(v_in_b[0:2 * E])
    biases[:, 8:16] = _col(h_in_b[0:2 * E])
    biases[:, 24:28] = _col(c1)
    biases[:, 28:32] = _col(mlp_b2)

    shared = {
        "w_vin": np.ascontiguousarray(v_in_w.T).astype(NPBF),
        "w_hq": np.ascontiguousarray(h_in_w[0:E].T).astype(NPBF),
        "w_hkv": np.ascontiguousarray(h_in_w[E:3 * E].T).astype(NPBF),
        "w_fh": np.ascontiguousarray(f_h.T).astype(NPBF),
        "w_fv": np.ascontiguousarray(f_v.T).astype(NPBF),
        "w_m2": np.ascontiguousarray(mlp_w2.T).astype(NPBF),
        "mask": _band_masks(),
        "biases": biases,
    }

    in_maps = []
    for c in range(NCORE):
        rows = x[RPC * c:RPC * (c + 1)]                      # (24, 192, 512)
        cols = x[:, RPC * c:RPC * (c + 1)].transpose(1, 0, 2)  # (24, 192, 512)
        m = dict(shared)
        m["xr_t"] = np.ascontiguousarray(rows.reshape(T, E).T).astype(NPBF)
        m["xc_t"] = np.ascontiguousarray(cols.reshape(T, E).T).astype(NPBF)
        in_maps.append(m)

    global _LAST_IN_MAPS
    _LAST_IN_MAPS = in_maps
    res = run_bass_kernel_spmd(nc, in_maps, core_ids=list(range(NCORE)))

    out = np.empty((S, S, E), np.float32)
    for c in range(NCORE):
        out[RPC * c:RPC * (c + 1)] = res.results[c]["out_t"].T.reshape(RPC, S, E)
    return out


# revision 51
# speedup vs baseline: 1.0046x; 1.0046x over previous
"""BiSPA (bidirectional sparse windowed attention + MLP) Trainium2 kernel.

Full inputs in, full outputs out. Internally shards across 8 NeuronCores:
core c owns output rows Ic = [24c, 24c+24) of the (192, 192, 512) grid.

Key observation: with B == S == 192 and window W == 32,
  - vertical attention for output row i is a complete 192-token sliding-window
    attention over x[i, :, :]                        (needs x rows  Ic)
  - horizontal attention for output row i is a complete 192-token
    sliding-window attention with Q from x[i, :, :] and K/V from x[:, i, :]
    (needs x columns Ic)
so each core needs x[Ic, :, :] and x[:, Ic, :] and NOTHING else -> zero
duplicated projection FLOPs, zero collectives, no halos.

Optimizations vs the first working kernel (tensor-engine bound, 864us
measured; this version measures ~446us):
  - out-projections folded into MLP1 on the host: F_h = W1h @ Who,
    F_v = W1v @ Wvo, c1 = W1h@h_out_eff + W1v@v_out_eff + b1. Removes
    384 N=384 matmuls + 8 evictions per pair.
  - disjoint q-split attention: TA covers q<96 (keys 0:128 via va), TB
    covers q>=96 (keys 64:192 via vb) -> TWO attn@V matmuls per head
    writing DISJOINT PSUM cells (no accumulate-overlap, so the PE
    pipelines them at ~45ns instead of serializing on the drain at
    ~160ns), and the TA score matmul shrinks from N=128 to N=96.
    NOTE the has_written trap: the group's start=True matmul must span
    every partition row the group writes, else a fresh bank's stale
    has_written bits make later matmuls accumulate power-on garbage.
  - scores for a head pair land in one 2-bank PSUM tile -> ONE exp
    activation per head pair ([128, 2, 192] strided read); probs of TWO
    head pairs share one SBUF tile -> ONE mask-multiply per 4 heads.
  - softmax normalization: one batched reciprocal ([96,2,2,1] AP over
    the 4 Z columns) + ONE tensor_tensor with a 0-stride broadcast AP
    of 1/Z, replacing 4 tensor_scalars.
  - ctx^T transposes stay on the PE but write bf16 into SPARE COLUMNS
    (f32 cols 260:356, bitcast) of the same PSUM bank as the attn@V
    accumulator, so no extra PSUM banks are needed; one copy per head
    pair (alternating ACT/DVE) evicts to SBUF.
  - emission software-pipelined: QK projections of pair g+1 are fed in
    8-matmul chunks EXACTLY at the points where the PE would otherwise
    stall on the DVE normalize latency (between each head-pair-pair's
    attn@V matmuls and its transposes). This keeps the PE array duty
    high everywhere, so HAM never re-throttles the clock (354us of the
    baseline ran at 1.2 GHz instead of 2.4 GHz). MLP of pair g is
    emitted during pair g+1 (or fed chunk-wise into the last pair), so
    it never waits on fresh ctx evictions.
  - PSUM budget: psA(proj) 2 banks + psS(scores) 2x2 banks + psC(ctx)
    2 banks = 8 banks exactly.

Numerics: all matmul inputs bf16, fp32 PSUM accumulation, fp32 softmax exp
input, bf16 probs/ctx.  Measured end-to-end rel err vs fp32 reference ~0.6%.
"""

import numpy as np
from contextlib import ExitStack

import concourse.bass as bass
import concourse.mybir as mybir
import concourse.tile as tile
from concourse import bacc
from concourse.bass_utils import run_bass_kernel_spmd
from concourse.masks import make_identity
from concourse.tile import add_dep_helper


def _chain(insts):
    """Order matmuls targeting one PSUM bank: a start=True zeroes (marks
    pending-zero) the WHOLE 2KB bank, so each bank must hold exactly one
    accumulation group and the group's matmuls must execute in program order.
    Tile won't order disjoint-region writes by itself."""
    for a, b in zip(insts, insts[1:]):
        add_dep_helper(b.ins, a.ins, sync=False, reason="psum-bank group order")

BF = mybir.dt.bfloat16
F32 = mybir.dt.float32
AF = mybir.ActivationFunctionType
MUL = mybir.AluOpType.mult
NPBF = mybir.dt.np(BF)

E = 512
H = 8
D = 64
W = 32
S = 192
NCORE = 8
RPC = 24          # rows (strips) per core
T = RPC * S       # tokens per core per branch = 4608
NPAIR = RPC // 2  # strip pairs per core


def _band_masks():
    """Score mask, bf16 (128, 384): [TA 96 | TB 96] x 2 heads.

    TA: rows p = key k in [0,128), cols q in [0,96).
        valid = |k-q| <= W              (every key for q<96 is in [0,128))
    TB: rows p -> key k = 64+p in [64,192), cols q' -> q = 96+q' in [96,192).
        valid = |k-q| <= W              (every key for q>=96 is in [64,192))

    q-ranges are disjoint between TA and TB, so the three attn@V matmuls
    per head write DISJOINT PSUM cells (no accumulate-overlap -> the PE
    pipelines them instead of serializing on the drain).
    """
    m = np.zeros((128, 192), np.float32)
    k = np.arange(128)[:, None]
    q = np.arange(96)[None, :]
    m[:, 0:96] = np.abs(k - q) <= W
    kb = 64 + np.arange(128)[:, None]
    qb = 96 + np.arange(96)[None, :]
    m[:, 96:192] = np.abs(kb - qb) <= W
    m2 = np.concatenate([m, m], axis=1)          # two heads of one p
    # [p0: 384 | pad 128 | p1: 384 | pad 128] matching the [128, 2, 512]
    # probs tiles, so ONE masked-multiply covers a whole head-pair-pair
    z = np.zeros((128, 128), np.float32)
    return np.concatenate([m2, z, m2, z], axis=1).astype(NPBF)


def _build_program(bias_flags):
    """Build the SPMD Bass/Tile program (same program on all 8 cores)."""
    has_vqk_b, has_hq_b, has_hk_b, has_c1, has_b2 = bias_flags

    nc = bacc.Bacc("TRN2", target_bir_lowering=False, debug=False,
                   num_devices=NCORE, num_swdge_queues=4)

    xr_t = nc.dram_tensor("xr_t", [E, T], BF, kind="ExternalInput").ap()
    xc_t = nc.dram_tensor("xc_t", [E, T], BF, kind="ExternalInput").ap()
    w_vin = nc.dram_tensor("w_vin", [E, 3 * E], BF, kind="ExternalInput").ap()
    w_hq = nc.dram_tensor("w_hq", [E, E], BF, kind="ExternalInput").ap()
    w_hkv = nc.dram_tensor("w_hkv", [E, 2 * E], BF, kind="ExternalInput").ap()
    w_fh = nc.dram_tensor("w_fh", [E, E], BF, kind="ExternalInput").ap()
    w_fv = nc.dram_tensor("w_fv", [E, E], BF, kind="ExternalInput").ap()
    w_m2 = nc.dram_tensor("w_m2", [E, E], BF, kind="ExternalInput").ap()
    mask_d = nc.dram_tensor("mask", [128, 1024], BF, kind="ExternalInput").ap()
    bias_d = nc.dram_tensor("biases", [128, 32], F32, kind="ExternalInput").ap()
    out_t = nc.dram_tensor("out_t", [E, T], F32, kind="ExternalOutput").ap()

    with tile.TileContext(nc) as tc, ExitStack() as ctx:
        pw = ctx.enter_context(tc.tile_pool(name="pw", bufs=1))
        psA = ctx.enter_context(tc.tile_pool(name="psA", bufs=2, space="PSUM"))
        psS = ctx.enter_context(tc.tile_pool(name="psS", bufs=2, space="PSUM"))
        psC = ctx.enter_context(tc.tile_pool(name="psC", bufs=2, space="PSUM"))
        px = ctx.enter_context(tc.tile_pool(name="px", bufs=3))
        pqk = ctx.enter_context(tc.tile_pool(name="pqk", bufs=32))
        pv = ctx.enter_context(tc.tile_pool(name="pv", bufs=8))
        pp = ctx.enter_context(tc.tile_pool(name="pp", bufs=12))
        pctx = ctx.enter_context(tc.tile_pool(name="pctx", bufs=8))
        pzr = ctx.enter_context(tc.tile_pool(name="pzr", bufs=8))
        pct = ctx.enter_context(tc.tile_pool(name="pct", bufs=16))
        phid = ctx.enter_context(tc.tile_pool(name="phid", bufs=8))
        pout = ctx.enter_context(tc.tile_pool(name="pout", bufs=8))

        # ---- persistent constants (loaded lazily in the emission section
        # so pair-0's activations and the QK-path weights reach SBUF first,
        # split across the gpsimd and sync DMA queues) ----
        def load_const(name, dram_ap, shape, dtype, eng):
            t = pw.tile(shape, dtype, tag=name)
            eng.dma_start(t[:], dram_ap)
            return t

        ident = pw.tile([128, 128], BF, tag="ident")
        make_identity(nc, ident)

        # bias column map (within `bia`):
        # 0-7 v_in_b[0:1024] ftiles; 8-11 h_in_b[0:512]; 12-15 h_in_b[512:1024]
        # 24-27 c1 (fused W1@out_bias + mlp_b1); 28-31 mlp_b2

        # per-pair state, filled by the emit helpers
        xr2_all = [None] * NPAIR
        xc2_all = [None] * NPAIR
        qk_all = [None] * NPAIR
        ct_all = [None] * NPAIR
        # ACT evictions of past cp tiles (psC bufs=2): the bitcast APs used
        # by the transposes/evict may not be seen by Tile's overlap tracker,
        # so the next occupant of the same PSUM buffer syncs explicitly.
        cp_evicts = []

        def emit_dma(g):
            g0 = 2 * S * g
            xr2, xc2 = [], []
            for k in range(4):
                t = px.tile([128, 2 * S], BF, tag=f"xr{k}")
                nc.gpsimd.dma_start(t[:], xr_t[128 * k:128 * (k + 1), g0:g0 + 2 * S])
                xr2.append(t)
                t = px.tile([128, 2 * S], BF, tag=f"xc{k}")
                nc.sync.dma_start(t[:], xc_t[128 * k:128 * (k + 1), g0:g0 + 2 * S])
                xc2.append(t)
            xr2_all[g] = xr2
            xc2_all[g] = xc2

        def emit_qk_ftile(g, branch, j):
            """One QK projection ftile for pair g, feature-major, N=384.
            ftile j in 0..7: j<4 -> Q features, j>=4 -> K features."""
            xr2, xc2 = xr2_all[g], xc2_all[g]
            ps = psA.tile([128, 384], F32, tag="proj",
                          padded_shape=[128, 512])
            for k in range(4):
                if branch == "v":
                    lhsT = wv[k][:, 128 * j:128 * (j + 1)]
                    rhs = xr2[k][:]
                elif j < 4:   # h Q
                    lhsT = whq[k][:, 128 * j:128 * (j + 1)]
                    rhs = xr2[k][:]
                else:         # h K
                    lhsT = whkv[k][:, 128 * (j - 4):128 * (j - 3)]
                    rhs = xc2[k][:]
                nc.tensor.matmul(ps[:], lhsT=lhsT, rhs=rhs,
                                 start=(k == 0), stop=(k == 3))
            # bias columns: v ftiles 0-7 -> cols 0-7; h Q 0-3 -> 8-11;
            # h K 0-3 -> 12-15. Evictions alternate ACT/DVE so neither
            # engine's FIFO latency stalls the psA buffer rotation.
            dst = pqk.tile([128, 384], BF, tag="qk")
            need_b = (has_vqk_b if branch == "v"
                      else (has_hq_b if j < 4 else has_hk_b))
            if need_b:
                bcol = j if branch == "v" else (8 + j)
                nc.scalar.activation(dst[:], ps[:], AF.Identity,
                                     bias=bia[:, bcol:bcol + 1])
            elif j % 2 == 0:
                nc.scalar.activation(dst[:], ps[:], AF.Copy)
            else:
                nc.vector.tensor_copy(dst[:], ps[:])
            qk_all[g][branch][j] = dst

        def emit_qk(g, branch):
            if qk_all[g] is None:
                qk_all[g] = {}
            qk_all[g][branch] = [None] * 8
            for j in range(8):
                emit_qk_ftile(g, branch, j)

        def qk_chunks(g):
            """8 chunk-emitters covering pair g's QK projections (2 ftiles
            = 8 matmuls each), to be interleaved into pair g-1's attention
            at the points where the PE would otherwise stall on the DVE
            normalize latency."""
            if qk_all[g] is None:
                qk_all[g] = {}
            chunks = []
            for branch in ("h", "v"):
                qk_all[g][branch] = [None] * 8
                for jj in (0, 2, 4, 6):
                    def mk(branch=branch, jj=jj):
                        emit_qk_ftile(g, branch, jj)
                        emit_qk_ftile(g, branch, jj + 1)
                    chunks.append(mk)
            return chunks

        def emit_attn_sb(g, a, br, feed):
            """Attention for strip a (0/1) of pair g, branch br. `feed()`
            emits a chunk of dense matmuls at the two points where the PE
            stream would otherwise stall on DVE normalize latency."""
            s0 = S * a
            xin = xr2_all[g] if br == "v" else xc2_all[g]
            vcols = slice(1024, 1536) if br == "v" else slice(512, 1024)
            vw = wv if br == "v" else whkv
            qk = qk_all[g][br]
            ct = ct_all[g]

            # ------ V projection, token-major, with ones column ----
            vps_a = psA.tile([128, 512], F32, tag="proj")
            vps_b = psA.tile([128, 512], F32, tag="proj")
            for k in range(4):
                nc.tensor.matmul(vps_a[:], lhsT=xin[k][:, s0:s0 + 128],
                                 rhs=vw[k][:, vcols],
                                 start=(k == 0), stop=(k == 3))
            for k in range(4):
                nc.tensor.matmul(vps_b[:], lhsT=xin[k][:, s0 + 64:s0 + 192],
                                 rhs=vw[k][:, vcols],
                                 start=(k == 0), stop=(k == 3))
            va = pv.tile([128, 8, 65], BF, tag="vp")   # keys [0:128)
            vb = pv.tile([128, 8, 65], BF, tag="vp")   # keys [64:192)
            nc.vector.tensor_copy(
                va[:, :, 0:64],
                vps_a[:].rearrange("p (h c) -> p h c", c=64))
            nc.vector.tensor_copy(
                vb[:, :, 0:64],
                vps_b[:].rearrange("p (h c) -> p h c", c=64))
            nc.vector.memset(va[:, :, 64:65], 1.0)
            nc.vector.memset(vb[:, :, 64:65], 1.0)

            # ------ scores + exp + mask, head pairs processed two at a
            # time sharing one probs tile -> ONE masked-multiply per four
            # heads ------
            pms = []
            for pg2 in (0, 2):
                pbp = pp.tile([128, 2, 512], BF, tag="p")
                pmp = pp.tile([128, 2, 512], BF, tag="p")
                for i, p in enumerate((pg2, pg2 + 1)):
                    QT = qk[p][:, s0:s0 + S]
                    KT = qk[4 + p][:, s0:s0 + S]
                    # one 2-bank tile per head pair; head h2 -> bank h2.
                    # Matmuls with disjoint contraction row-groups (head0
                    # at partitions 0:64, head1 at 64:128) run CONCURRENTLY
                    # on the PE and hard-fault if they write the same bank.
                    sp = psS.tile([128, 2, 512], F32, tag="sc")
                    for h2 in range(2):
                        d0 = 64 * h2
                        nc.tensor.matmul(sp[:, h2:h2 + 1, 0:96],
                                         lhsT=KT[d0:d0 + 64, 0:128],
                                         rhs=QT[d0:d0 + 64, 0:96],
                                         start=True, stop=True)
                        nc.tensor.matmul(sp[:, h2:h2 + 1, 96:192],
                                         lhsT=KT[d0:d0 + 64, 64:192],
                                         rhs=QT[d0:d0 + 64, 96:192],
                                         start=True, stop=True)
                    # ONE exp for both heads: strided [128, 2, 192] read
                    nc.scalar.activation(
                        pbp[:, i:i + 1, 0:384]
                        .rearrange("p a (b c) -> p (a b) c", b=2),
                        sp[:, :, 0:192], AF.Exp, scale=0.125)
                nc.vector.tensor_tensor(pmp[:, :, 0:384], pbp[:, :, 0:384],
                                        msk[:, 0:1024]
                                        .rearrange("p (i c) -> p i c", i=2)
                                        [:, :, 0:384], op=MUL)
                pms.append((pmp, 0))
                pms.append((pmp, 1))

            # ------ attn@V + normalize + ctx transpose per head pair ------
            # p-groups are processed two at a time with their matmuls
            # interleaved across the two psC banks so drains overlap.
            for pg in (0, 2):
                group = []
                for p in (pg, pg + 1):
                    pmp, pi = pms[p]
                    # attn@V: TWO matmuls per head (q 0:96 via va, q 96:192
                    # via vb), writing DISJOINT cells of one PSUM bank, all
                    # within partitions 0:96. The built-in group checker
                    # cannot express multi-region banks, so skip it;
                    # correctness comes from the issue ordering +
                    # per-element pending-zero semantics (mm1's start=True
                    # clears the bank rows this group ever touches).
                    cp = psC.tile([128, 512], F32, tag="cx")
                    mms = []
                    for h2 in range(2):
                        h = 2 * p + h2
                        cb = 130 * h2
                        ta = 192 * h2
                        tb = 192 * h2 + 96
                        # q in [0,96): keys [0:128) from TA
                        mms.append(nc.tensor.matmul(
                            cp[0:96, cb:cb + 65],
                            lhsT=pmp[:, pi:pi + 1, ta:ta + 96],
                            rhs=va[:, h:h + 1, :], start=(h2 == 0),
                            stop=False, skip_group_check=True))
                        # q in [96,192): keys [64:192) = all of TB
                        mms.append(nc.tensor.matmul(
                            cp[0:96, cb + 65:cb + 130],
                            lhsT=pmp[:, pi:pi + 1, tb:tb + 96],
                            rhs=vb[:, h:h + 1, :],
                            start=False, stop=(h2 == 1),
                            skip_group_check=True))
                    _chain(mms)
                    group.append((p, cp, mms))

                norm_done = []
                for p, cp, mms in group:
                    # normalize by 1/Z (Z = ones-column accumulation at cols
                    # {64, 129, 194, 259} = 64 + 130h + 65q) and pack for
                    # the transpose: ctxn = [q1: h0|h1, q2: h0|h1], 64 cols
                    # each, on partitions 0:96 (q1 = tok 0:96, q2 = 96:192)
                    zr = pzr.tile([96, 2, 2, 1], F32, tag="zr")
                    zin = (cp[0:96, 64:324]
                           .rearrange("p (h x) -> p h x", h=2)
                           .rearrange("p h (q c) -> p h q c", q=2))
                    ctxn = pctx.tile([96, 256], BF, tag="ctxn")
                    cpq = (cp[0:96, 0:260]
                           .rearrange("p (h x) -> p h x", h=2)
                           .rearrange("p h (q c) -> p h q c", q=2))
                    cto = ctxn[:].rearrange("p (q h c) -> p h q c", q=2, h=2)
                    reads = [
                        nc.vector.reciprocal(zr[:], zin[:, :, :, 0:1]),
                        nc.vector.tensor_tensor(
                            cto, cpq[:, :, :, 0:64],
                            zr[:, :, :, 0:1].broadcast_to([96, 2, 2, 64]),
                            op=MUL),
                    ]
                    # cp reads must wait for the accumulation group to close
                    # (same-bank PE-write + DVE-read is a HW fault)
                    for r in reads:
                        add_dep_helper(r.ins, mms[-1].ins, sync=True,
                                       reason="psum read after group close")
                    norm_done.append((p, cp, mms, ctxn, reads))

                # dense matmuls here: they keep the PE streaming while the
                # DVE normalize drains, so the transposes never stall the
                # PE FIFO (head-of-line).
                feed()

                for p, cp, mms, ctxn, reads in norm_done:
                    # ctx^T via PE transpose into the SPARE columns
                    # (260:356) of the same PSUM bank -- no extra banks.
                    tps = [
                        nc.tensor.transpose(cp[:, 260:308].bitcast(BF),
                                            ctxn[:, 0:128],
                                            ident[0:96, 0:96]),
                        nc.tensor.transpose(cp[:, 308:356].bitcast(BF),
                                            ctxn[:, 128:256],
                                            ident[0:96, 0:96]),
                    ]
                    for t in tps:
                        for r in reads:
                            add_dep_helper(t.ins, r.ins, sync=True,
                                           reason="transpose after reads")
                    ct_p = ct[(0 if br == "h" else 4) + p]
                    if p % 2 == 0:
                        ev = nc.scalar.activation(
                            ct_p[:, s0:s0 + S],
                            cp[:, 260:356].bitcast(BF), AF.Copy)
                    else:
                        ev = nc.vector.tensor_copy(
                            ct_p[:, s0:s0 + S],
                            cp[:, 260:356].bitcast(BF))
                    for t in tps:
                        add_dep_helper(ev.ins, t.ins, sync=True,
                                       reason="evict after transpose")
                    cp_evicts.append(ev)
                    if len(cp_evicts) >= 3:
                        add_dep_helper(mms[0].ins, cp_evicts[-3].ins,
                                       sync=True,
                                       reason="cp buffer reuse after evict")

        def emit_mlp_hid(g, j, hid):
            ct = ct_all[g]
            ps = psA.tile([128, 384], F32, tag="proj",
                          padded_shape=[128, 512])
            for k in range(4):
                nc.tensor.matmul(
                    ps[:], lhsT=wfh[k][:, 128 * j:128 * (j + 1)],
                    rhs=ct[k][:], start=(k == 0), stop=False)
            for k in range(4):
                nc.tensor.matmul(
                    ps[:], lhsT=wfv[k][:, 128 * j:128 * (j + 1)],
                    rhs=ct[4 + k][:], start=False, stop=(k == 3))
            dst = phid.tile([128, 384], BF, tag="hid")
            if has_c1:
                nc.scalar.activation(dst[:], ps[:], AF.Relu,
                                     bias=bia[:, 24 + j:24 + j + 1])
            else:
                nc.scalar.activation(dst[:], ps[:], AF.Relu)
            hid[j] = dst

        def emit_mlp_out(g, j, hid):
            g0 = 2 * S * g
            ps = psA.tile([128, 384], F32, tag="proj",
                          padded_shape=[128, 512])
            for k in range(4):
                nc.tensor.matmul(ps[:],
                                 lhsT=wm2[k][:, 128 * j:128 * (j + 1)],
                                 rhs=hid[k][:],
                                 start=(k == 0), stop=(k == 3))
            osb = pout.tile([128, 384], F32, tag="o")
            if has_b2:
                nc.scalar.activation(osb[:], ps[:], AF.Identity,
                                     bias=bia[:, 28 + j:28 + j + 1])
            else:
                nc.scalar.activation(osb[:], ps[:], AF.Copy)
            nc.sync.dma_start(out_t[128 * j:128 * (j + 1), g0:g0 + 2 * S],
                              osb[:])

        def emit_mlp(g):
            """Fused (out-proj + MLP1) then MLP2 for pair g, N=384."""
            hid = [None] * 4
            for j in range(4):
                emit_mlp_hid(g, j, hid)
            for j in range(4):
                emit_mlp_out(g, j, hid)

        def mlp_chunks(g):
            """8 chunk-emitters covering pair g's MLP, for interleaving
            into the final pair's attention (which has no QK to feed)."""
            hid = [None] * 4
            chunks = []
            for j in range(4):
                chunks.append(lambda j=j: emit_mlp_hid(g, j, hid))
            for j in range(4):
                chunks.append(lambda j=j: emit_mlp_out(g, j, hid))
            return chunks

        # ---- software-pipelined emission ----
        # attention of pair g interleaves with QK projections of pair g+1
        # (dense N=384 matmuls) so the PE array duty stays high and HAM
        # keeps the 2.4 GHz clock.
        emit_dma(0)
        whq = [load_const(f"whq{k}", w_hq[128 * k:128 * (k + 1), :],
                          [128, E], BF,
                          nc.gpsimd if k % 2 == 0 else nc.sync)
               for k in range(4)]
        whkv = [load_const(f"whkv{k}", w_hkv[128 * k:128 * (k + 1), :],
                           [128, 2 * E], BF,
                           nc.gpsimd if k % 2 == 0 else nc.sync)
                for k in range(4)]
        wv = [load_const(f"wv{k}", w_vin[128 * k:128 * (k + 1), :],
                         [128, 3 * E], BF,
                         nc.gpsimd if k % 2 == 0 else nc.sync)
              for k in range(4)]
        msk = load_const("msk", mask_d[:, :], [128, 1024], BF, nc.sync)
        bia = load_const("bia", bias_d[:, :], [128, 32], F32, nc.sync)
        emit_dma(1)
        wfh = [load_const(f"wfh{k}", w_fh[128 * k:128 * (k + 1), :],
                          [128, E], BF, nc.sync)
               for k in range(4)]
        wfv = [load_const(f"wfv{k}", w_fv[128 * k:128 * (k + 1), :],
                          [128, E], BF, nc.sync)
               for k in range(4)]
        wm2 = [load_const(f"wm2{k}", w_m2[128 * k:128 * (k + 1), :],
                          [128, E], BF, nc.sync)
               for k in range(4)]
        for g in range(NPAIR):
            ct_all[g] = [pct.tile([128, 2 * S], BF, tag="ct",
                                  name=f"ct_{g}_{i}") for i in range(8)]
            if g == 0:
                emit_qk(0, "h")
                emit_qk(0, "v")
            if g + 2 < NPAIR:
                emit_dma(g + 2)
            # For the last pair there is no QK to feed, so feed the
            # previous pair's MLP chunks into the stall points instead.
            last = g + 1 >= NPAIR
            chunks = mlp_chunks(g - 1) if last else qk_chunks(g + 1)
            ci = [0]

            def feed():
                if ci[0] < len(chunks):
                    chunks[ci[0]]()
                    ci[0] += 1
            for a in range(2):
                emit_attn_sb(g, a, "h", feed)
                emit_attn_sb(g, a, "v", feed)
                # MLP of the PREVIOUS pair: its ctx evictions are long done,
                # so these dense matmuls never stall the PE stream.
                if a == 1 and g > 0 and not last:
                    emit_mlp(g - 1)
        emit_mlp(NPAIR - 1)
    nc.finalize()
    return nc


_CACHE = {}


def _get_program(bias_flags):
    key = tuple(bias_flags)
    if key not in _CACHE:
        _CACHE[key] = _build_program(key)
    return _CACHE[key]


def _col(b):
    """bias vector (128*n,) -> (128, n) column-pack, fortran-ish layout."""
    return np.ascontiguousarray(b.reshape(-1, 128).T.astype(np.float32))


def kernel(hidden_states, h_in_w, h_in_b, h_out_w, h_out_b,
           v_in_w, v_in_b, v_out_w, v_out_b,
           mlp_w1, mlp_b1, mlp_w2, mlp_b2):
    x = np.asarray(hidden_states, dtype=np.float32)
    h_in_w = np.asarray(h_in_w, np.float32)
    h_in_b = np.asarray(h_in_b, np.float32)
    h_out_w = np.asarray(h_out_w, np.float32)
    h_out_b = np.asarray(h_out_b, np.float32)
    v_in_w = np.asarray(v_in_w, np.float32)
    v_in_b = np.asarray(v_in_b, np.float32)
    v_out_w = np.asarray(v_out_w, np.float32)
    v_out_b = np.asarray(v_out_b, np.float32)
    mlp_w1 = np.asarray(mlp_w1, np.float32)
    mlp_b1 = np.asarray(mlp_b1, np.float32)
    mlp_w2 = np.asarray(mlp_w2, np.float32)
    mlp_b2 = np.asarray(mlp_b2, np.float32)

    # V biases act as a constant shift of ctx (softmax weights sum to 1),
    # so fold them through the out-projections; then fold the
    # out-projections themselves into MLP1 (everything is linear up to the
    # ReLU): hid = relu(W1h(Who ctx_h + bho) + W1v(Wvo ctx_v + bvo) + b1)
    #            = relu(F_h ctx_h + F_v ctx_v + c1)
    h_out_eff = h_out_b + h_out_w @ h_in_b[2 * E:3 * E]
    v_out_eff = v_out_b + v_out_w @ v_in_b[2 * E:3 * E]
    w1h = mlp_w1[:, 0:E]
    w1v = mlp_w1[:, E:2 * E]
    f_h = w1h @ h_out_w          # (E, E): hid += F_h @ ctx_h
    f_v = w1v @ v_out_w
    c1 = w1h @ h_out_eff + w1v @ v_out_eff + mlp_b1

    bias_flags = (
        bool(np.any(v_in_b[0:2 * E])), bool(np.any(h_in_b[0:E])),
        bool(np.any(h_in_b[E:2 * E])), bool(np.any(c1)), bool(np.any(mlp_b2)),
    )
    nc = _get_program(bias_flags)

    biases = np.zeros((128, 32), np.float32)
    biases[:, 0:8] = _col(v_in_b[0:2 * E])
    biases[:, 8:16] = _col(h_in_b[0:2 * E])
    biases[:, 24:28] = _col(c1)
    biases[:, 28:32] = _col(mlp_b2)

    shared = {
        "w_vin": np.ascontiguousarray(v_in_w.T).astype(NPBF),
        "w_hq": np.ascontiguousarray(h_in_w[0:E].T).astype(NPBF),
        "w_hkv": np.ascontiguousarray(h_in_w[E:3 * E].T).astype(NPBF),
        "w_fh": np.ascontiguousarray(f_h.T).astype(NPBF),
        "w_fv": np.ascontiguousarray(f_v.T).astype(NPBF),
        "w_m2": np.ascontiguousarray(mlp_w2.T).astype(NPBF),
        "mask": _band_masks(),
        "biases": biases,
    }

    in_maps = []
    for c in range(NCORE):
        rows = x[RPC * c:RPC * (c + 1)]                      # (24, 192, 512)
        cols = x[:, RPC * c:RPC * (c + 1)].transpose(1, 0, 2)  # (24, 192, 512)
        m = dict(shared)
        m["xr_t"] = np.ascontiguousarray(rows.reshape(T, E).T).astype(NPBF)
        m["xc_t"] = np.ascontiguousarray(cols.reshape(T, E).T).astype(NPBF)
        in_maps.append(m)

    global _LAST_IN_MAPS
    _LAST_IN_MAPS = in_maps
    res = run_bass_kernel_spmd(nc, in_maps, core_ids=list(range(NCORE)))

    out = np.empty((S, S, E), np.float32)
    for c in range(NCORE):
        out[RPC * c:RPC * (c + 1)] = res.results[c]["out_t"].T.reshape(RPC, S, E)
    return out


# revision 53
# speedup vs baseline: 1.0066x; 1.0020x over previous
"""BiSPA (bidirectional sparse windowed attention + MLP) Trainium2 kernel.

Full inputs in, full outputs out. Internally shards across 8 NeuronCores:
core c owns output rows Ic = [24c, 24c+24) of the (192, 192, 512) grid.

Key observation: with B == S == 192 and window W == 32,
  - vertical attention for output row i is a complete 192-token sliding-window
    attention over x[i, :, :]                        (needs x rows  Ic)
  - horizontal attention for output row i is a complete 192-token
    sliding-window attention with Q from x[i, :, :] and K/V from x[:, i, :]
    (needs x columns Ic)
so each core needs x[Ic, :, :] and x[:, Ic, :] and NOTHING else -> zero
duplicated projection FLOPs, zero collectives, no halos.

Optimizations vs the first working kernel (tensor-engine bound, 864us
measured; this version measures ~446us):
  - out-projections folded into MLP1 on the host: F_h = W1h @ Who,
    F_v = W1v @ Wvo, c1 = W1h@h_out_eff + W1v@v_out_eff + b1. Removes
    384 N=384 matmuls + 8 evictions per pair.
  - disjoint q-split attention: TA covers q<96 (keys 0:128 via va), TB
    covers q>=96 (keys 64:192 via vb) -> TWO attn@V matmuls per head
    writing DISJOINT PSUM cells (no accumulate-overlap, so the PE
    pipelines them at ~45ns instead of serializing on the drain at
    ~160ns), and the TA score matmul shrinks from N=128 to N=96.
    NOTE the has_written trap: the group's start=True matmul must span
    every partition row the group writes, else a fresh bank's stale
    has_written bits make later matmuls accumulate power-on garbage.
  - scores for a head pair land in one 2-bank PSUM tile -> ONE exp
    activation per head pair ([128, 2, 192] strided read); probs of TWO
    head pairs share one SBUF tile -> ONE mask-multiply per 4 heads.
  - softmax normalization: one batched reciprocal ([96,2,2,1] AP over
    the 4 Z columns) + ONE tensor_tensor with a 0-stride broadcast AP
    of 1/Z, replacing 4 tensor_scalars.
  - ctx^T transposes stay on the PE but write bf16 into SPARE COLUMNS
    (f32 cols 260:356, bitcast) of the same PSUM bank as the attn@V
    accumulator, so no extra PSUM banks are needed; one copy per head
    pair (alternating ACT/DVE) evicts to SBUF.
  - emission software-pipelined: QK projections of pair g+1 are fed in
    8-matmul chunks EXACTLY at the points where the PE would otherwise
    stall on the DVE normalize latency (between each head-pair-pair's
    attn@V matmuls and its transposes). This keeps the PE array duty
    high everywhere, so HAM never re-throttles the clock (354us of the
    baseline ran at 1.2 GHz instead of 2.4 GHz). MLP of pair g is
    emitted during pair g+1 (or fed chunk-wise into the last pair), so
    it never waits on fresh ctx evictions.
  - PSUM budget: psA(proj) 2 banks + psS(scores) 2x2 banks + psC(ctx)
    2 banks = 8 banks exactly.

Numerics: all matmul inputs bf16, fp32 PSUM accumulation, fp32 softmax exp
input, bf16 probs/ctx.  Measured end-to-end rel err vs fp32 reference ~0.6%.
"""

import numpy as np
from contextlib import ExitStack

import concourse.bass as bass
import concourse.mybir as mybir
import concourse.tile as tile
from concourse import bacc
from concourse.bass_utils import run_bass_kernel_spmd
from concourse.masks import make_identity
from concourse.tile import add_dep_helper


def _chain(insts):
    """Order matmuls targeting one PSUM bank: a start=True zeroes (marks
    pending-zero) the WHOLE 2KB bank, so each bank must hold exactly one
    accumulation group and the group's matmuls must execute in program order.
    Tile won't order disjoint-region writes by itself."""
    for a, b in zip(insts, insts[1:]):
        add_dep_helper(b.ins, a.ins, sync=False, reason="psum-bank group order")

BF = mybir.dt.bfloat16
F32 = mybir.dt.float32
AF = mybir.ActivationFunctionType
MUL = mybir.AluOpType.mult
NPBF = mybir.dt.np(BF)

E = 512
H = 8
D = 64
W = 32
S = 192
NCORE = 8
RPC = 24          # rows (strips) per core
T = RPC * S       # tokens per core per branch = 4608
NPAIR = RPC // 2  # strip pairs per core


def _band_masks():
    """Score mask, bf16 (128, 384): [TA 96 | TB 96] x 2 heads.

    TA: rows p = key k in [0,128), cols q in [0,96).
        valid = |k-q| <= W              (every key for q<96 is in [0,128))
    TB: rows p -> key k = 64+p in [64,192), cols q' -> q = 96+q' in [96,192).
        valid = |k-q| <= W              (every key for q>=96 is in [64,192))

    q-ranges are disjoint between TA and TB, so the three attn@V matmuls
    per head write DISJOINT PSUM cells (no accumulate-overlap -> the PE
    pipelines them instead of serializing on the drain).
    """
    m = np.zeros((128, 192), np.float32)
    k = np.arange(128)[:, None]
    q = np.arange(96)[None, :]
    m[:, 0:96] = np.abs(k - q) <= W
    kb = 64 + np.arange(128)[:, None]
    qb = 96 + np.arange(96)[None, :]
    m[:, 96:192] = np.abs(kb - qb) <= W
    m2 = np.concatenate([m, m], axis=1)          # two heads of one p
    # [p0: 384 | pad 128 | p1: 384 | pad 128] matching the [128, 2, 512]
    # probs tiles, so ONE masked-multiply covers a whole head-pair-pair
    z = np.zeros((128, 128), np.float32)
    return np.concatenate([m2, z, m2, z], axis=1).astype(NPBF)


def _build_program(bias_flags):
    """Build the SPMD Bass/Tile program (same program on all 8 cores)."""
    has_vqk_b, has_hq_b, has_hk_b, has_c1, has_b2 = bias_flags

    nc = bacc.Bacc("TRN2", target_bir_lowering=False, debug=False,
                   num_devices=NCORE, num_swdge_queues=4)

    xr_t = nc.dram_tensor("xr_t", [E, T], BF, kind="ExternalInput").ap()
    xc_t = nc.dram_tensor("xc_t", [E, T], BF, kind="ExternalInput").ap()
    w_vin = nc.dram_tensor("w_vin", [E, 3 * E], BF, kind="ExternalInput").ap()
    w_hq = nc.dram_tensor("w_hq", [E, E], BF, kind="ExternalInput").ap()
    w_hkv = nc.dram_tensor("w_hkv", [E, 2 * E], BF, kind="ExternalInput").ap()
    w_fh = nc.dram_tensor("w_fh", [E, E], BF, kind="ExternalInput").ap()
    w_fv = nc.dram_tensor("w_fv", [E, E], BF, kind="ExternalInput").ap()
    w_m2 = nc.dram_tensor("w_m2", [E, E], BF, kind="ExternalInput").ap()
    mask_d = nc.dram_tensor("mask", [128, 1024], BF, kind="ExternalInput").ap()
    bias_d = nc.dram_tensor("biases", [128, 32], F32, kind="ExternalInput").ap()
    out_t = nc.dram_tensor("out_t", [E, T], F32, kind="ExternalOutput").ap()

    with tile.TileContext(nc) as tc, ExitStack() as ctx:
        pw = ctx.enter_context(tc.tile_pool(name="pw", bufs=1))
        psA = ctx.enter_context(tc.tile_pool(name="psA", bufs=2, space="PSUM"))
        psS = ctx.enter_context(tc.tile_pool(name="psS", bufs=2, space="PSUM"))
        psC = ctx.enter_context(tc.tile_pool(name="psC", bufs=2, space="PSUM"))
        px = ctx.enter_context(tc.tile_pool(name="px", bufs=3))
        pqk = ctx.enter_context(tc.tile_pool(name="pqk", bufs=32))
        pv = ctx.enter_context(tc.tile_pool(name="pv", bufs=8))
        pp = ctx.enter_context(tc.tile_pool(name="pp", bufs=12))
        pctx = ctx.enter_context(tc.tile_pool(name="pctx", bufs=8))
        pzr = ctx.enter_context(tc.tile_pool(name="pzr", bufs=8))
        pct = ctx.enter_context(tc.tile_pool(name="pct", bufs=16))
        phid = ctx.enter_context(tc.tile_pool(name="phid", bufs=8))
        pout = ctx.enter_context(tc.tile_pool(name="pout", bufs=8))

        # ---- persistent constants (loaded lazily in the emission section
        # so pair-0's activations and the QK-path weights reach SBUF first,
        # split across the gpsimd and sync DMA queues) ----
        def load_const(name, dram_ap, shape, dtype, eng):
            t = pw.tile(shape, dtype, tag=name)
            eng.dma_start(t[:], dram_ap)
            return t

        ident = pw.tile([128, 128], BF, tag="ident")
        make_identity(nc, ident)

        # bias column map (within `bia`):
        # 0-7 v_in_b[0:1024] ftiles; 8-11 h_in_b[0:512]; 12-15 h_in_b[512:1024]
        # 24-27 c1 (fused W1@out_bias + mlp_b1); 28-31 mlp_b2

        # per-pair state, filled by the emit helpers
        xr2_all = [None] * NPAIR
        xc2_all = [None] * NPAIR
        qk_all = [None] * NPAIR
        ct_all = [None] * NPAIR
        # ACT evictions of past cp tiles (psC bufs=2): the bitcast APs used
        # by the transposes/evict may not be seen by Tile's overlap tracker,
        # so the next occupant of the same PSUM buffer syncs explicitly.
        cp_evicts = []

        def emit_dma(g):
            g0 = 2 * S * g
            xr2, xc2 = [], []
            for k in range(4):
                t = px.tile([128, 2 * S], BF, tag=f"xr{k}")
                nc.gpsimd.dma_start(t[:], xr_t[128 * k:128 * (k + 1), g0:g0 + 2 * S])
                xr2.append(t)
                t = px.tile([128, 2 * S], BF, tag=f"xc{k}")
                nc.sync.dma_start(t[:], xc_t[128 * k:128 * (k + 1), g0:g0 + 2 * S])
                xc2.append(t)
            xr2_all[g] = xr2
            xc2_all[g] = xc2

        def emit_qk_ftile(g, branch, j):
            """One QK projection ftile for pair g, feature-major, N=384.
            ftile j in 0..7: j<4 -> Q features, j>=4 -> K features."""
            xr2, xc2 = xr2_all[g], xc2_all[g]
            ps = psA.tile([128, 384], F32, tag="proj",
                          padded_shape=[128, 512])
            for k in range(4):
                if branch == "v":
                    lhsT = wv[k][:, 128 * j:128 * (j + 1)]
                    rhs = xr2[k][:]
                elif j < 4:   # h Q
                    lhsT = whq[k][:, 128 * j:128 * (j + 1)]
                    rhs = xr2[k][:]
                else:         # h K
                    lhsT = whkv[k][:, 128 * (j - 4):128 * (j - 3)]
                    rhs = xc2[k][:]
                nc.tensor.matmul(ps[:], lhsT=lhsT, rhs=rhs,
                                 start=(k == 0), stop=(k == 3))
            # bias columns: v ftiles 0-7 -> cols 0-7; h Q 0-3 -> 8-11;
            # h K 0-3 -> 12-15. Evictions alternate ACT/DVE so neither
            # engine's FIFO latency stalls the psA buffer rotation.
            dst = pqk.tile([128, 384], BF, tag="qk")
            need_b = (has_vqk_b if branch == "v"
                      else (has_hq_b if j < 4 else has_hk_b))
            if need_b:
                bcol = j if branch == "v" else (8 + j)
                nc.scalar.activation(dst[:], ps[:], AF.Identity,
                                     bias=bia[:, bcol:bcol + 1])
            elif j % 2 == 0:
                nc.scalar.activation(dst[:], ps[:], AF.Copy)
            else:
                nc.vector.tensor_copy(dst[:], ps[:])
            qk_all[g][branch][j] = dst

        def emit_qk(g, branch):
            if qk_all[g] is None:
                qk_all[g] = {}
            qk_all[g][branch] = [None] * 8
            for j in range(8):
                emit_qk_ftile(g, branch, j)

        def qk_chunks(g):
            """8 chunk-emitters covering pair g's QK projections (2 ftiles
            = 8 matmuls each), to be interleaved into pair g-1's attention
            at the points where the PE would otherwise stall on the DVE
            normalize latency."""
            if qk_all[g] is None:
                qk_all[g] = {}
            chunks = []
            for branch in ("h", "v"):
                qk_all[g][branch] = [None] * 8
                for jj in (0, 2, 4, 6):
                    def mk(branch=branch, jj=jj):
                        emit_qk_ftile(g, branch, jj)
                        emit_qk_ftile(g, branch, jj + 1)
                    chunks.append(mk)
            return chunks

        def emit_attn_sb(g, a, br, feed):
            """Attention for strip a (0/1) of pair g, branch br. `feed()`
            emits a chunk of dense matmuls at the two points where the PE
            stream would otherwise stall on DVE normalize latency."""
            s0 = S * a
            xin = xr2_all[g] if br == "v" else xc2_all[g]
            vcols = slice(1024, 1536) if br == "v" else slice(512, 1024)
            vw = wv if br == "v" else whkv
            qk = qk_all[g][br]
            ct = ct_all[g]

            # ------ scores + exp + mask, head pairs processed two at a
            # time sharing one probs tile -> ONE masked-multiply per four
            # heads. Scores go FIRST (they use psS, not psA) so the psA
            # evictions of the preceding feed chunk get ~700ns more slack
            # before the V projection needs those buffers. ------
            pms = []
            for pg2 in (0, 2):
                pbp = pp.tile([128, 2, 512], BF, tag="p")
                pmp = pp.tile([128, 2, 512], BF, tag="p")
                for i, p in enumerate((pg2, pg2 + 1)):
                    QT = qk[p][:, s0:s0 + S]
                    KT = qk[4 + p][:, s0:s0 + S]
                    # one 2-bank tile per head pair; head h2 -> bank h2.
                    # Matmuls with disjoint contraction row-groups (head0
                    # at partitions 0:64, head1 at 64:128) run CONCURRENTLY
                    # on the PE and hard-fault if they write the same bank.
                    sp = psS.tile([128, 2, 512], F32, tag="sc")
                    for h2 in range(2):
                        d0 = 64 * h2
                        nc.tensor.matmul(sp[:, h2:h2 + 1, 0:96],
                                         lhsT=KT[d0:d0 + 64, 0:128],
                                         rhs=QT[d0:d0 + 64, 0:96],
                                         start=True, stop=True)
                        nc.tensor.matmul(sp[:, h2:h2 + 1, 96:192],
                                         lhsT=KT[d0:d0 + 64, 64:192],
                                         rhs=QT[d0:d0 + 64, 96:192],
                                         start=True, stop=True)
                    # ONE exp for both heads: strided [128, 2, 192] read
                    nc.scalar.activation(
                        pbp[:, i:i + 1, 0:384]
                        .rearrange("p a (b c) -> p (a b) c", b=2),
                        sp[:, :, 0:192], AF.Exp, scale=0.125)
                nc.vector.tensor_tensor(pmp[:, :, 0:384], pbp[:, :, 0:384],
                                        msk[:, 0:1024]
                                        .rearrange("p (i c) -> p i c", i=2)
                                        [:, :, 0:384], op=MUL)
                pms.append((pmp, 0))
                pms.append((pmp, 1))

            # ------ V projection, token-major, with ones column ----
            vps_a = psA.tile([128, 512], F32, tag="proj")
            vps_b = psA.tile([128, 512], F32, tag="proj")
            for k in range(4):
                nc.tensor.matmul(vps_a[:], lhsT=xin[k][:, s0:s0 + 128],
                                 rhs=vw[k][:, vcols],
                                 start=(k == 0), stop=(k == 3))
            for k in range(4):
                nc.tensor.matmul(vps_b[:], lhsT=xin[k][:, s0 + 64:s0 + 192],
                                 rhs=vw[k][:, vcols],
                                 start=(k == 0), stop=(k == 3))
            va = pv.tile([128, 8, 65], BF, tag="vp")   # keys [0:128)
            vb = pv.tile([128, 8, 65], BF, tag="vp")   # keys [64:192)
            nc.vector.tensor_copy(
                va[:, :, 0:64],
                vps_a[:].rearrange("p (h c) -> p h c", c=64))
            nc.vector.tensor_copy(
                vb[:, :, 0:64],
                vps_b[:].rearrange("p (h c) -> p h c", c=64))
            nc.vector.memset(va[:, :, 64:65], 1.0)
            nc.vector.memset(vb[:, :, 64:65], 1.0)

            # ------ attn@V + normalize + ctx transpose per head pair ------
            # p-groups are processed two at a time with their matmuls
            # interleaved across the two psC banks so drains overlap.
            for pg in (0, 2):
                group = []
                for p in (pg, pg + 1):
                    pmp, pi = pms[p]
                    # attn@V: TWO matmuls per head (q 0:96 via va, q 96:192
                    # via vb), writing DISJOINT cells of one PSUM bank, all
                    # within partitions 0:96. The built-in group checker
                    # cannot express multi-region banks, so skip it;
                    # correctness comes from the issue ordering +
                    # per-element pending-zero semantics (mm1's start=True
                    # clears the bank rows this group ever touches).
                    cp = psC.tile([128, 512], F32, tag="cx")
                    mms = []
                    for h2 in range(2):
                        h = 2 * p + h2
                        cb = 130 * h2
                        ta = 192 * h2
                        tb = 192 * h2 + 96
                        # q in [0,96): keys [0:128) from TA
                        mms.append(nc.tensor.matmul(
                            cp[0:96, cb:cb + 65],
                            lhsT=pmp[:, pi:pi + 1, ta:ta + 96],
                            rhs=va[:, h:h + 1, :], start=(h2 == 0),
                            stop=False, skip_group_check=True))
                        # q in [96,192): keys [64:192) = all of TB
                        mms.append(nc.tensor.matmul(
                            cp[0:96, cb + 65:cb + 130],
                            lhsT=pmp[:, pi:pi + 1, tb:tb + 96],
                            rhs=vb[:, h:h + 1, :],
                            start=False, stop=(h2 == 1),
                            skip_group_check=True))
                    _chain(mms)
                    group.append((p, cp, mms))

                norm_done = []
                for p, cp, mms in group:
                    # normalize by 1/Z (Z = ones-column accumulation at cols
                    # {64, 129, 194, 259} = 64 + 130h + 65q) and pack for
                    # the transpose: ctxn = [q1: h0|h1, q2: h0|h1], 64 cols
                    # each, on partitions 0:96 (q1 = tok 0:96, q2 = 96:192)
                    zr = pzr.tile([96, 2, 2, 1], F32, tag="zr")
                    zin = (cp[0:96, 64:324]
                           .rearrange("p (h x) -> p h x", h=2)
                           .rearrange("p h (q c) -> p h q c", q=2))
                    ctxn = pctx.tile([96, 256], BF, tag="ctxn")
                    cpq = (cp[0:96, 0:260]
                           .rearrange("p (h x) -> p h x", h=2)
                           .rearrange("p h (q c) -> p h q c", q=2))
                    cto = ctxn[:].rearrange("p (q h c) -> p h q c", q=2, h=2)
                    reads = [
                        nc.vector.reciprocal(zr[:], zin[:, :, :, 0:1]),
                        nc.vector.tensor_tensor(
                            cto, cpq[:, :, :, 0:64],
                            zr[:, :, :, 0:1].broadcast_to([96, 2, 2, 64]),
                            op=MUL),
                    ]
                    # cp reads must wait for the accumulation group to close
                    # (same-bank PE-write + DVE-read is a HW fault)
                    for r in reads:
                        add_dep_helper(r.ins, mms[-1].ins, sync=True,
                                       reason="psum read after group close")
                    norm_done.append((p, cp, mms, ctxn, reads))

                # dense matmuls here: they keep the PE streaming while the
                # DVE normalize drains, so the transposes never stall the
                # PE FIFO (head-of-line).
                feed()

                for p, cp, mms, ctxn, reads in norm_done:
                    # ctx^T via PE transpose into the SPARE columns
                    # (260:356) of the same PSUM bank -- no extra banks.
                    tps = [
                        nc.tensor.transpose(cp[:, 260:308].bitcast(BF),
                                            ctxn[:, 0:128],
                                            ident[0:96, 0:96]),
                        nc.tensor.transpose(cp[:, 308:356].bitcast(BF),
                                            ctxn[:, 128:256],
                                            ident[0:96, 0:96]),
                    ]
                    for t in tps:
                        for r in reads:
                            add_dep_helper(t.ins, r.ins, sync=True,
                                           reason="transpose after reads")
                    ct_p = ct[(0 if br == "h" else 4) + p]
                    if p % 2 == 0:
                        ev = nc.scalar.activation(
                            ct_p[:, s0:s0 + S],
                            cp[:, 260:356].bitcast(BF), AF.Copy)
                    else:
                        ev = nc.vector.tensor_copy(
                            ct_p[:, s0:s0 + S],
                            cp[:, 260:356].bitcast(BF))
                    for t in tps:
                        add_dep_helper(ev.ins, t.ins, sync=True,
                                       reason="evict after transpose")
                    cp_evicts.append(ev)
                    if len(cp_evicts) >= 3:
                        add_dep_helper(mms[0].ins, cp_evicts[-3].ins,
                                       sync=True,
                                       reason="cp buffer reuse after evict")

        def emit_mlp_hid(g, j, hid):
            ct = ct_all[g]
            ps = psA.tile([128, 384], F32, tag="proj",
                          padded_shape=[128, 512])
            for k in range(4):
                nc.tensor.matmul(
                    ps[:], lhsT=wfh[k][:, 128 * j:128 * (j + 1)],
                    rhs=ct[k][:], start=(k == 0), stop=False)
            for k in range(4):
                nc.tensor.matmul(
                    ps[:], lhsT=wfv[k][:, 128 * j:128 * (j + 1)],
                    rhs=ct[4 + k][:], start=False, stop=(k == 3))
            dst = phid.tile([128, 384], BF, tag="hid")
            if has_c1:
                nc.scalar.activation(dst[:], ps[:], AF.Relu,
                                     bias=bia[:, 24 + j:24 + j + 1])
            else:
                nc.scalar.activation(dst[:], ps[:], AF.Relu)
            hid[j] = dst

        def emit_mlp_out(g, j, hid):
            g0 = 2 * S * g
            ps = psA.tile([128, 384], F32, tag="proj",
                          padded_shape=[128, 512])
            for k in range(4):
                nc.tensor.matmul(ps[:],
                                 lhsT=wm2[k][:, 128 * j:128 * (j + 1)],
                                 rhs=hid[k][:],
                                 start=(k == 0), stop=(k == 3))
            osb = pout.tile([128, 384], F32, tag="o")
            if has_b2:
                nc.scalar.activation(osb[:], ps[:], AF.Identity,
                                     bias=bia[:, 28 + j:28 + j + 1])
            else:
                nc.scalar.activation(osb[:], ps[:], AF.Copy)
            nc.sync.dma_start(out_t[128 * j:128 * (j + 1), g0:g0 + 2 * S],
                              osb[:])

        def emit_mlp(g):
            """Fused (out-proj + MLP1) then MLP2 for pair g, N=384."""
            hid = [None] * 4
            for j in range(4):
                emit_mlp_hid(g, j, hid)
            for j in range(4):
                emit_mlp_out(g, j, hid)

        def mlp_chunks(g):
            """8 chunk-emitters covering pair g's MLP, for interleaving
            into the final pair's attention (which has no QK to feed)."""
            hid = [None] * 4
            chunks = []
            for j in range(4):
                chunks.append(lambda j=j: emit_mlp_hid(g, j, hid))
            for j in range(4):
                chunks.append(lambda j=j: emit_mlp_out(g, j, hid))
            return chunks

        # ---- software-pipelined emission ----
        # attention of pair g interleaves with QK projections of pair g+1
        # (dense N=384 matmuls) so the PE array duty stays high and HAM
        # keeps the 2.4 GHz clock.
        emit_dma(0)
        whq = [load_const(f"whq{k}", w_hq[128 * k:128 * (k + 1), :],
                          [128, E], BF,
                          nc.gpsimd if k % 2 == 0 else nc.sync)
               for k in range(4)]
        whkv = [load_const(f"whkv{k}", w_hkv[128 * k:128 * (k + 1), :],
                           [128, 2 * E], BF,
                           nc.gpsimd if k % 2 == 0 else nc.sync)
                for k in range(4)]
        wv = [load_const(f"wv{k}", w_vin[128 * k:128 * (k + 1), :],
                         [128, 3 * E], BF,
                         nc.gpsimd if k % 2 == 0 else nc.sync)
              for k in range(4)]
        msk = load_const("msk", mask_d[:, :], [128, 1024], BF, nc.sync)
        bia = load_const("bia", bias_d[:, :], [128, 32], F32, nc.sync)
        emit_dma(1)
        wfh = [load_const(f"wfh{k}", w_fh[128 * k:128 * (k + 1), :],
                          [128, E], BF, nc.sync)
               for k in range(4)]
        wfv = [load_const(f"wfv{k}", w_fv[128 * k:128 * (k + 1), :],
                          [128, E], BF, nc.sync)
               for k in range(4)]
        wm2 = [load_const(f"wm2{k}", w_m2[128 * k:128 * (k + 1), :],
                          [128, E], BF, nc.sync)
               for k in range(4)]
        for g in range(NPAIR):
            ct_all[g] = [pct.tile([128, 2 * S], BF, tag="ct",
                                  name=f"ct_{g}_{i}") for i in range(8)]
            if g == 0:
                emit_qk(0, "h")
                emit_qk(0, "v")
            if g + 2 < NPAIR:
                emit_dma(g + 2)
            # For the last pair there is no QK to feed, so feed the
            # previous pair's MLP chunks into the stall points instead.
            last = g + 1 >= NPAIR
            chunks = mlp_chunks(g - 1) if last else qk_chunks(g + 1)
            ci = [0]

            def feed():
                if ci[0] < len(chunks):
                    chunks[ci[0]]()
                    ci[0] += 1
            for a in range(2):
                emit_attn_sb(g, a, "h", feed)
                emit_attn_sb(g, a, "v", feed)
                # MLP of the PREVIOUS pair: its ctx evictions are long done,
                # so these dense matmuls never stall the PE stream.
                if a == 1 and g > 0 and not last:
                    emit_mlp(g - 1)
        emit_mlp(NPAIR - 1)
    nc.finalize()
    return nc


_CACHE = {}


def _get_program(bias_flags):
    key = tuple(bias_flags)
    if key not in _CACHE:
        _CACHE[key] = _build_program(key)
    return _CACHE[key]


def _col(b):
    """bias vector (128*n,) -> (128, n) column-pack, fortran-ish layout."""
    return np.ascontiguousarray(b.reshape(-1, 128).T.astype(np.float32))


def kernel(hidden_states, h_in_w, h_in_b, h_out_w, h_out_b,
           v_in_w, v_in_b, v_out_w, v_out_b,
           mlp_w1, mlp_b1, mlp_w2, mlp_b2):
    x = np.asarray(hidden_states, dtype=np.float32)
    h_in_w = np.asarray(h_in_w, np.float32)
    h_in_b = np.asarray(h_in_b, np.float32)
    h_out_w = np.asarray(h_out_w, np.float32)
    h_out_b = np.asarray(h_out_b, np.float32)
    v_in_w = np.asarray(v_in_w, np.float32)
    v_in_b = np.asarray(v_in_b, np.float32)
    v_out_w = np.asarray(v_out_w, np.float32)
    v_out_b = np.asarray(v_out_b, np.float32)
    mlp_w1 = np.asarray(mlp_w1, np.float32)
    mlp_b1 = np.asarray(mlp_b1, np.float32)
    mlp_w2 = np.asarray(mlp_w2, np.float32)
    mlp_b2 = np.asarray(mlp_b2, np.float32)

    # V biases act as a constant shift of ctx (softmax weights sum to 1),
    # so fold them through the out-projections; then fold the
    # out-projections themselves into MLP1 (everything is linear up to the
    # ReLU): hid = relu(W1h(Who ctx_h + bho) + W1v(Wvo ctx_v + bvo) + b1)
    #            = relu(F_h ctx_h + F_v ctx_v + c1)
    h_out_eff = h_out_b + h_out_w @ h_in_b[2 * E:3 * E]
    v_out_eff = v_out_b + v_out_w @ v_in_b[2 * E:3 * E]
    w1h = mlp_w1[:, 0:E]
    w1v = mlp_w1[:, E:2 * E]
    f_h = w1h @ h_out_w          # (E, E): hid += F_h @ ctx_h
    f_v = w1v @ v_out_w
    c1 = w1h @ h_out_eff + w1v @ v_out_eff + mlp_b1

    bias_flags = (
        bool(np.any(v_in_b[0:2 * E])), bool(np.any(h_in_b[0:E])),
        bool(np.any(h_in_b[E:2 * E])), bool(np.any(c1)), bool(np.any(mlp_b2)),
    )
    nc = _get_program(bias_flags)

    biases = np.zeros((128, 32), np.float32)
    biases[:, 0:8] = _col(v_in_b[0:2 * E])
    biases[:, 8:16] = _col(h_in_b[0:2 * E])
    biases[:, 24:28] = _col(c1)
    biases[:, 28:32] = _col(mlp_b2)

    shared = {
        "w_vin": np.ascontiguousarray(v_in_w.T).astype(NPBF),
        "w_hq": np.ascontiguousarray(h_in_w[0:E].T).astype(NPBF),
        "w_hkv": np.ascontiguousarray(h_in_w[E:3 * E].T).astype(NPBF),
        "w_fh": np.ascontiguousarray(f_h.T).astype(NPBF),
        "w_fv": np.ascontiguousarray(f_v.T).astype(NPBF),
        "w_m2": np.ascontiguousarray(mlp_w2.T).astype(NPBF),
        "mask": _band_masks(),
        "biases": biases,
    }

    in_maps = []
    for c in range(NCORE):
        rows = x[RPC * c:RPC * (c + 1)]                      # (24, 192, 512)
        cols = x[:, RPC * c:RPC * (c + 1)].transpose(1, 0, 2)  # (24, 192, 512)
        m = dict(shared)
        m["xr_t"] = np.ascontiguousarray(rows.reshape(T, E).T).astype(NPBF)
        m["xc_t"] = np.ascontiguousarray(cols.reshape(T, E).T).astype(NPBF)
        in_maps.append(m)

    global _LAST_IN_MAPS
    _LAST_IN_MAPS = in_maps
    res = run_bass_kernel_spmd(nc, in_maps, core_ids=list(range(NCORE)))

    out = np.empty((S, S, E), np.float32)
    for c in range(NCORE):
        out[RPC * c:RPC * (c + 1)] = res.results[c]["out_t"].T.reshape(RPC, S, E)
    return out
